# revision 1
# baseline (speedup 1.0000x reference)
"""BoundaryLoss kernel for 8 Trainium2 NeuronCores.

loss = sum_c mean_{b,h,w}((|sobel(labels_c)| - |sobel(probs_c)|)^2)
     = sum_sq_err / (B*H*W)

Data-parallel: core k processes batches [2k, 2k+1] x classes 1..4
(8 image pairs of 512x512). Per-core partial sums are combined on host.

On-device pipeline per (pair, row-band):
  - DMA 128-row halo band of labels + probs image (fp32, padded cols).
  - TensorE: gx = Bv @ x[w-1] - Bv @ x[w+1]; gy = Bdf @ (x[w-1] + 2x[w] + x[w+1])
    via 5 float32r band-matrix matmuls per input accumulating in PSUM.
  - ScalarE/VectorE: square PSUM -> fp16, m = gx^2+gy^2, G = sqrt(m+eps),
    e = G_l - G_p, then tensor_tensor_reduce(e*e) -> per-band partial sums.
"""

import sys

import numpy as np

if "/opt/trn_rl_repo" not in sys.path:
    sys.path.insert(0, "/opt/trn_rl_repo")

from contextlib import ExitStack

import concourse.bass as bass
import concourse.mybir as mybir
import concourse.tile as tile

H = W = 512
N_IMG = 8          # image pairs per core
BAND = 126         # output rows per full band
N_BANDS = 4        # full 126-row bands; bottom 8 rows via 2 packed iters
N_ITERS = N_IMG * N_BANDS + 2
PADW = W + 2       # padded columns
SMOOTH = 1e-6
# columns of the 2048-wide PSUM square handled by ScalarE (rest on VectorE)
ACT_SQ_COLS = 1696

F32 = mybir.dt.float32
F32R = mybir.dt.float32r
F16 = mybir.dt.float16


def _band_geom(t):
    """Returns (row0, nrows_loaded, dst_part0, n_valid_out, n_contract)."""
    if t == 0:
        return 0, 127, 1, BAND, 128
    if t < 4:
        r0 = BAND * t
        return r0 - 1, 128, 0, BAND, 128
    # kp=9: row 512 (would be partition 9) is simply dropped from the
    # contraction, which equals the zero-padding contribution.
    return 503, 9, 0, 8, 9


def _stationaries():
    """lhsT weight matrices [p, c]: moving partition p -> out partition c."""
    bv = np.zeros((128, 128), np.float32)   # vertical smooth [1,2,1]
    bdf = np.zeros((128, 128), np.float32)  # vertical diff [1,0,-1]
    for c in range(126):
        bv[c, c] = 1.0
        bv[c + 1, c] = 2.0
        bv[c + 2, c] = 1.0
        bdf[c, c] = 1.0
        bdf[c + 2, c] = -1.0
    # Packed bottom-band versions: 4 images per iteration; image k's rows
    # 503..511 live at input partitions 16k..16k+8 (16k+9 is the zeroed
    # row-512 halo), outputs 504..511 at partitions 8k..8k+7.
    bvm = np.zeros((128, 128), np.float32)
    bdfm = np.zeros((128, 128), np.float32)
    for k in range(4):
        for i in range(8):
            bvm[16 * k + i, 8 * k + i] = 1.0
            bvm[16 * k + i + 1, 8 * k + i] = 2.0
            bvm[16 * k + i + 2, 8 * k + i] = 1.0
            bdfm[16 * k + i, 8 * k + i] = 1.0
            bdfm[16 * k + i + 2, 8 * k + i] = -1.0
    return np.concatenate(
        [bv, -bv, bdf, 2.0 * bdf, bvm, -bvm, bdfm, 2.0 * bdfm],
        axis=1).astype(np.float16)


def _split_waits_json(bir: bytes, maxw: int = 1) -> bytes:
    """Walrus in this container rejects instructions with >1 semaphore wait
    ("Too many sync wait commands"). Split extra waits onto NoOp carriers
    inserted just before the instruction on the same engine — semantics are
    identical (same waits, same order, before the instruction executes)."""
    import orjson

    d = orjson.loads(bir)
    ctr = 0
    for fn in d["functions"]:
        for b in fn["blocks"]:
            new = []
            for ins in b["instructions"]:
                si = ins.get("sync_info")
                if si:
                    waits = si.get("on_wait") or []
                    if len(waits) > maxw:
                        keep = waits[-maxw:] if maxw else []
                        for w in waits[: len(waits) - maxw]:
                            ctr += 1
                            new.append({
                                "debug": ins.get("debug", 0),
                                "engine": ins["engine"],
                                "ins": [],
                                "outs": [],
                                "name": f"{ins['name']}-wsplit{ctr}",
                                "opcode": "NoOp",
                                "sync_info": {"on_wait": [w], "on_update": []},
                            })
                        si["on_wait"] = keep
                new.append(ins)
            b["instructions"] = new
    return orjson.dumps(d)


def _patch_serialization(nc):
    fixed = _split_waits_json(nc.to_json_bytes())
    nc.to_json_bytes = lambda: fixed
    return nc


def build_kernel(loop: int = 1):
    nc = bass.Bass()
    labels = nc.dram_tensor("labels", [N_IMG, H, W], F16, kind="ExternalInput")
    probs = nc.dram_tensor("probs", [N_IMG, H, W], F16, kind="ExternalInput")
    consts = nc.dram_tensor("consts", [128, 1024], F16, kind="ExternalInput")
    out = nc.dram_tensor("out", [128, 3], F32, kind="ExternalOutput")

    with ExitStack() as ctx:
        tc = ctx.enter_context(tile.TileContext(nc))
        cpool = ctx.enter_context(tc.tile_pool(name="consts", bufs=1))
        xpool = ctx.enter_context(tc.tile_pool(name="x", bufs=1))
        psum_pool = ctx.enter_context(tc.tile_pool(name="g", bufs=2, space="PSUM"))
        sq_pool = ctx.enter_context(tc.tile_pool(name="sq", bufs=4))
        m_pool = ctx.enter_context(tc.tile_pool(name="m", bufs=4))
        g2_pool = ctx.enter_context(tc.tile_pool(name="G", bufs=4))
        e_pool = ctx.enter_context(tc.tile_pool(name="e", bufs=4))
        esq_pool = ctx.enter_context(tc.tile_pool(name="esq", bufs=4))
        acc_pool = ctx.enter_context(tc.tile_pool(name="acc", bufs=1))

        wmat = cpool.tile([128, 1024], F16, tag="wmat")
        nc.sync.dma_start(out=wmat[:, :], in_=consts[:, :])
        (BV, BVN, BDF, BDF2, BVM, BVNM, BDFM, BDF2M) = (
            wmat[:, 128 * i:128 * i + 128] for i in range(8))

        acc_a = acc_pool.tile([128, N_ITERS], F32, tag="acc_a")
        acc_b = acc_pool.tile([128, N_ITERS], F32, tag="acc_b")
        acc_c = acc_pool.tile([128, N_ITERS], F32, tag="acc_c")
        nc.vector.memset(acc_a[:, :], 0.0)
        nc.vector.memset(acc_b[:, :], 0.0)
        nc.vector.memset(acc_c[:, :], 0.0)
        out_s = acc_pool.tile([128, 3], F32, tag="out_s")

        # 8 persistent x tiles; band t always lands on tiles {2t, 2t+1}.
        # Pad regions are zeroed once and never overwritten (the DMAs fill
        # the interior only).
        xt = [xpool.tile([128, PADW], F16, name=f"x{j}", tag=f"x{j}")
              for j in range(8)]
        for j in range(8):
            nc.vector.memset(xt[j][:, 0:1], 0.0)
            nc.vector.memset(xt[j][:, PADW - 1:PADW], 0.0)
        for j in (0, 1):
            nc.vector.memset(xt[j][0:1, :], 0.0)   # top band: row -1
        # 4 tiles for the packed bottom-band iterations (2 per input side).
        xm = [xpool.tile([128, PADW], F16, name=f"xm{j}", tag=f"xm{j}")
              for j in range(4)]
        for j in range(4):
            nc.vector.memset(xm[j][0:64, :], 0.0)

        loop_ctx = tc.For_i(0, loop, 1) if loop > 1 else None
        if loop_ctx is not None:
            loop_ctx.__enter__()

        def emit_mms(g, xlr, xpr, stat, pv, kp):
            # Stationary-major order: 4 weight loads per iteration, not 10.
            sv, svn, sdf, sdf2 = stat
            xs = ((xlr, 0), (xpr, 1024))
            for x, c in xs:
                nc.tensor.matmul(g[0:pv, c:c + 512], sv[0:kp, 0:pv],
                                 x[0:kp, 0:W], start=True, stop=False)
            for x, c in xs:
                nc.tensor.matmul(g[0:pv, c:c + 512], svn[0:kp, 0:pv],
                                 x[0:kp, 2:2 + W], start=False, stop=True)
            for x, c in xs:
                nc.tensor.matmul(g[0:pv, c + 512:c + 1024], sdf[0:kp, 0:pv],
                                 x[0:kp, 0:W], start=True, stop=False)
                nc.tensor.matmul(g[0:pv, c + 512:c + 1024], sdf[0:kp, 0:pv],
                                 x[0:kp, 2:2 + W], start=False, stop=False)
            for x, c in xs:
                nc.tensor.matmul(g[0:pv, c + 512:c + 1024], sdf2[0:kp, 0:pv],
                                 x[0:kp, 1:1 + W], start=False, stop=True)

        it = 0
        for phase in range(N_IMG + 2):
            if phase < N_IMG:
                img = phase
                bands = range(N_BANDS)
            else:
                bands = (-1,)
            for t in bands:
                if t >= 0:
                    r0, nrows, p0, pv, kp = _band_geom(t)
                    xlr, xpr = xt[2 * t], xt[2 * t + 1]
                    nc.sync.dma_start(
                        out=xlr[p0:p0 + nrows, 1:1 + W],
                        in_=labels[img, r0:r0 + nrows, :])
                    nc.sync.dma_start(
                        out=xpr[p0:p0 + nrows, 1:1 + W],
                        in_=probs[img, r0:r0 + nrows, :])
                    stat, pv, kp = (BV, BVN, BDF, BDF2), BAND, 128
                else:
                    # Packed bottom bands: rows 503..511 of 4 images.
                    q = phase - N_IMG
                    xlr, xpr = xm[2 * q], xm[2 * q + 1]
                    for k in range(4):
                        img_k = 4 * q + k
                        nc.sync.dma_start(
                            out=xlr[16 * k:16 * k + 9, 1:1 + W],
                            in_=labels[img_k, 503:512, :])
                        nc.sync.dma_start(
                            out=xpr[16 * k:16 * k + 9, 1:1 + W],
                            in_=probs[img_k, 503:512, :])
                    stat, pv, kp = (BVM, BVNM, BDFM, BDF2M), 32, 58

                # PSUM layout: [gx_l | gy_l | gx_p | gy_p], 512 f32 each.
                g = psum_pool.tile([128, 2048], F32)
                emit_mms(g, xlr, xpr, stat, pv, kp)

                # Squares of all four gradients, PSUM -> SBUF fp16. DVE
                # cannot read two PSUM operands in one op, so its share goes
                # through an fp16 copy. Sum(gx^2+gy^2) over both inputs is
                # captured for free by the accum_out of the ACT square and
                # the DVE TTR square. (SMOOTH inside the sqrt contributes
                # ~1e-7 relative to the loss and is dropped.)
                sq = sq_pool.tile([128, 2048], F16)
                nc.scalar.activation(sq[0:pv, 0:ACT_SQ_COLS],
                                     g[0:pv, 0:ACT_SQ_COLS],
                                     mybir.ActivationFunctionType.Square,
                                     accum_out=acc_a[0:pv, it:it + 1])
                dc = 2048 - ACT_SQ_COLS
                c16 = e_pool.tile([128, dc], F16)
                nc.vector.tensor_copy(c16[0:pv, :], g[0:pv, ACT_SQ_COLS:2048])
                nc.vector.scalar_tensor_tensor(
                    out=sq[0:pv, ACT_SQ_COLS:2048], in0=c16[0:pv, :],
                    scalar=1.0, in1=c16[0:pv, :],
                    op0=mybir.AluOpType.mult, op1=mybir.AluOpType.mult,
                    accum_out=acc_c[0:pv, it:it + 1])

                # m = gx^2 + gy^2 for both inputs: [m_l | m_p]
                m = m_pool.tile([128, 1024], F16)
                sqv = sq.rearrange("p (a b c) -> p a b c", a=2, b=2, c=512)
                mv = m.rearrange("p (a c) -> p a c", a=2, c=512)
                nc.vector.tensor_add(mv[0:pv, :, :], sqv[0:pv, :, 0, :],
                                     sqv[0:pv, :, 1, :])

                # (G_l - G_p)^2 = m_l + m_p - 2*sqrt(m_l * m_p)
                qp = g2_pool.tile([128, 512], F16)
                nc.vector.tensor_mul(qp[0:pv, :], m[0:pv, 0:512], m[0:pv, 512:1024])
                s = esq_pool.tile([128, 512], F16)
                nc.scalar.activation(s[0:pv, :], qp[0:pv, :],
                                     mybir.ActivationFunctionType.Sqrt,
                                     accum_out=acc_b[0:pv, it:it + 1])
                it += 1

        if loop_ctx is not None:
            loop_ctx.__exit__(None, None, None)
        nc.vector.tensor_reduce(out_s[:, 0:1], acc_a[:, :],
                                axis=mybir.AxisListType.X, op=mybir.AluOpType.add)
        nc.vector.tensor_reduce(out_s[:, 1:2], acc_b[:, :],
                                axis=mybir.AxisListType.X, op=mybir.AluOpType.add)
        nc.vector.tensor_reduce(out_s[:, 2:3], acc_c[:, :],
                                axis=mybir.AxisListType.X, op=mybir.AluOpType.add)
        nc.sync.dma_start(out=out[:, :], in_=out_s[:, :])
    return _patch_serialization(nc)


_NC = None


def kernel(probs, labels):
    global _NC
    from concourse.bass_utils import run_bass_kernel_spmd

    if _NC is None:
        _NC = build_kernel()

    p = np.ascontiguousarray(np.asarray(probs)[:, 1:5]).astype(np.float16)
    l = np.ascontiguousarray(np.asarray(labels)[:, 1:5]).astype(np.float16)
    wmat = _stationaries()

    in_maps = []
    for k in range(8):
        in_maps.append({
            "probs": np.ascontiguousarray(p[2 * k:2 * k + 2].reshape(N_IMG, H, W)),
            "labels": np.ascontiguousarray(l[2 * k:2 * k + 2].reshape(N_IMG, H, W)),
            "consts": wmat,
        })
    res = run_bass_kernel_spmd(_NC, in_maps, list(range(8)))
    total = 0.0
    for r in res.results:
        o = r["out"].astype(np.float64)
        total += o[:, 0].sum() + o[:, 2].sum() - 2.0 * o[:, 1].sum()
    return np.float32(total / (16 * H * W))



# revision 4
# speedup vs baseline: 1.0810x; 1.0810x over previous
"""BoundaryLoss kernel for 8 Trainium2 NeuronCores (v2).

loss = sum_c mean_{b,h,w}((|sobel(labels_c)| - |sobel(probs_c)|)^2)

Data-parallel: core k processes batches [2k, 2k+1] x classes 1..4
(8 image pairs of 512x512). Per-core partial sums are combined on host.

v2 structure (measured-cost-balanced):
  - Host packs labels+probs into one fp16 tensor x[8, 2, 512, 512]; all
    input DMAs issue at loop start into 8 persistent pair-tiles
    (4 row-band blocks x 2 sides, halo rows included per block).
  - TensorE: per band, 10 fp16 band-matrix matmuls -> PSUM
    [gx_l | gy_l | gx_p | gy_p] (4 x 512 f32).
  - ACT: one batched Square over PSUM cols [0:ACT_SQ_COLS] (+accum).
  - DVE: copy PSUM cols [ACT_SQ_COLS:2048] -> fp16, STT square (+accum).
  - DVE: m = gx^2+gy^2 (both sides, one TT), q = m_l*m_p into a 4-iter
    q-batch; madd/qmul lag one iteration behind eviction to hide the
    ACT->DVE dependency.
  - ACT: one Sqrt per 4 iters over the q-batch (+accum), emitted 2
    iterations late so it never stalls the Square stream.
  loss*B*H*W = sum(acc_a) + sum(acc_c) - 2*sum(acc_b), combined on host.
"""

import sys

import numpy as np

if "/opt/trn_rl_repo" not in sys.path:
    sys.path.insert(0, "/opt/trn_rl_repo")

from contextlib import ExitStack

import concourse.bass as bass
import concourse.mybir as mybir
import concourse.tile as tile

H = W = 512
N_IMG = 8          # image pairs per core
BAND = 126         # output rows per full band
N_BANDS = 4        # full 126-row bands; bottom 8 rows via 2 packed iters
N_ITERS = N_IMG * N_BANDS + 2
PADW = W + 2       # padded columns per block
BLK = PADW         # block stride inside a pair tile
SMOOTH = 1e-6
# columns of the 2048-wide PSUM handled by ACT Square (rest via DVE)
ACT_SQ_COLS = 1472
QB = 4             # iters per sqrt batch
SQRT_LAG = 2       # sqrt of batch b is emitted during iter 4*b+4+SQRT_LAG

F32 = mybir.dt.float32
F16 = mybir.dt.float16


def _stationaries():
    """lhsT weight matrices [p, c]: moving partition p -> out partition c."""
    bv = np.zeros((128, 128), np.float32)   # vertical smooth [1,2,1]
    bdf = np.zeros((128, 128), np.float32)  # vertical diff [1,0,-1]
    for c in range(126):
        bv[c, c] = 1.0
        bv[c + 1, c] = 2.0
        bv[c + 2, c] = 1.0
        bdf[c, c] = 1.0
        bdf[c + 2, c] = -1.0
    # Packed bottom-band versions: 4 images per iteration; image k's rows
    # 503..511 live at input partitions 16k..16k+8 (16k+9 is the zeroed
    # row-512 halo), outputs 504..511 at partitions 8k..8k+7.
    bvm = np.zeros((128, 128), np.float32)
    bdfm = np.zeros((128, 128), np.float32)
    for k in range(4):
        for i in range(8):
            bvm[16 * k + i, 8 * k + i] = 1.0
            bvm[16 * k + i + 1, 8 * k + i] = 2.0
            bvm[16 * k + i + 2, 8 * k + i] = 1.0
            bdfm[16 * k + i, 8 * k + i] = 1.0
            bdfm[16 * k + i + 2, 8 * k + i] = -1.0
    return np.concatenate(
        [bv, -bv, bdf, 2.0 * bdf, bvm, -bvm, bdfm, 2.0 * bdfm],
        axis=1).astype(np.float16)


def _split_waits_json(bir: bytes, maxw: int = 1) -> bytes:
    """Walrus in this container rejects instructions with >1 semaphore wait
    ("Too many sync wait commands"). Split extra waits onto NoOp carriers
    inserted just before the instruction on the same engine — semantics are
    identical (same waits, same order, before the instruction executes)."""
    import orjson

    d = orjson.loads(bir)
    ctr = 0
    for fn in d["functions"]:
        for b in fn["blocks"]:
            new = []
            for ins in b["instructions"]:
                si = ins.get("sync_info")
                if si:
                    waits = si.get("on_wait") or []
                    if len(waits) > maxw:
                        keep = waits[-maxw:] if maxw else []
                        for w in waits[: len(waits) - maxw]:
                            ctr += 1
                            new.append({
                                "debug": ins.get("debug", 0),
                                "engine": ins["engine"],
                                "ins": [],
                                "outs": [],
                                "name": f"{ins['name']}-wsplit{ctr}",
                                "opcode": "NoOp",
                                "sync_info": {"on_wait": [w], "on_update": []},
                            })
                        si["on_wait"] = keep
                new.append(ins)
            b["instructions"] = new
    return orjson.dumps(d)


def _patch_serialization(nc):
    fixed = _split_waits_json(nc.to_json_bytes())
    nc.to_json_bytes = lambda: fixed
    return nc


def build_kernel(loop: int = 1):
    nc = bass.Bass()
    x = nc.dram_tensor("x", [N_IMG, 2, H, W], F16, kind="ExternalInput")
    consts = nc.dram_tensor("consts", [128, 1024], F16, kind="ExternalInput")
    out = nc.dram_tensor("out", [128, 3], F32, kind="ExternalOutput")

    with ExitStack() as ctx:
        tc = ctx.enter_context(tile.TileContext(nc))
        cpool = ctx.enter_context(tc.tile_pool(name="consts", bufs=1))
        xpool = ctx.enter_context(tc.tile_pool(name="x", bufs=1))
        psum_pool = ctx.enter_context(tc.tile_pool(name="g", bufs=2, space="PSUM"))
        sq_pool = ctx.enter_context(tc.tile_pool(name="sq", bufs=4))
        c16_pool = ctx.enter_context(tc.tile_pool(name="c16", bufs=4))
        m_pool = ctx.enter_context(tc.tile_pool(name="m", bufs=4))
        q_pool = ctx.enter_context(tc.tile_pool(name="q", bufs=3))
        acc_pool = ctx.enter_context(tc.tile_pool(name="acc", bufs=1))

        wmat = cpool.tile([128, 1024], F16, tag="wmat")
        nc.sync.dma_start(out=wmat[:, :], in_=consts[:, :])
        (BV, BVN, BDF, BDF2, BVM, BVNM, BDFM, BDF2M) = (
            wmat[:, 128 * i:128 * i + 128] for i in range(8))

        acc_a = acc_pool.tile([128, N_ITERS], F32, tag="acc_a")
        acc_c = acc_pool.tile([128, N_ITERS], F32, tag="acc_c")
        acc_b = acc_pool.tile([128, N_ITERS // QB + 1], F32, tag="acc_b")
        nc.vector.memset(acc_a[:, :], 0.0)
        nc.vector.memset(acc_b[:, :], 0.0)
        nc.vector.memset(acc_c[:, :], 0.0)
        out_s = acc_pool.tile([128, 3], F32, tag="out_s")

        # Pair tiles: image pair i -> [128, 2 sides x 4 blocks x BLK].
        # Block (s, b) holds image rows 126b-1 .. 126b+126 of side s at
        # partitions 0..127 (partition p = row 126b-1+p), interior columns
        # 1..512; cols 0 and 513 stay zero (horizontal pad), and block 0
        # partition 0 stays zero (top halo row -1).
        pt = [xpool.tile([128, 2 * 4 * BLK], F16, name=f"pt{i}", tag=f"pt{i}")
              for i in range(N_IMG)]
        for i in range(N_IMG):
            nc.vector.memset(pt[i][:, :], 0.0)
        # Packed bottom-band tiles (2 iters x 2 sides).
        xm = [xpool.tile([128, PADW], F16, name=f"xm{j}", tag=f"xm{j}")
              for j in range(4)]
        for j in range(4):
            nc.vector.memset(xm[j][:, :], 0.0)

        def blk(i, s, b):
            """Full block AP [128, BLK] of pair i, side s, block b."""
            return pt[i][:, (4 * s + b) * BLK:(4 * s + b) * BLK + BLK]

        def emit_dmas():
            for i in range(N_IMG):
                # block 0: rows 0..126 -> partitions 1..127, both sides
                v0 = pt[i].rearrange("p (s b w) -> p s b w", s=2, b=4)
                nc.sync.dma_start(
                    out=v0[1:128, :, 0, 1:1 + W],
                    in_=x[i].rearrange("s p w -> p s w", p=H)[0:127, :, :])
                # blocks 1..3: rows 126b-1 .. 126b+126 -> partitions 0..127
                for b in range(1, 4):
                    r0 = 126 * b - 1
                    nc.sync.dma_start(
                        out=v0[:, :, b, 1:1 + W],
                        in_=x[i, :, r0:r0 + 128, :].rearrange(
                            "s p w -> p s w"))
            # packed bottom: rows 503..511 of image k at partitions
            # 16k..16k+8 of xm tile (q = k // 4 selects the iter pair)
            for q in range(2):
                for k in range(4):
                    img_k = 4 * q + k
                    nc.sync.dma_start(
                        out=xm[2 * q][16 * k:16 * k + 9, 1:1 + W],
                        in_=x[img_k, 0, 503:512, :])
                    nc.sync.dma_start(
                        out=xm[2 * q + 1][16 * k:16 * k + 9, 1:1 + W],
                        in_=x[img_k, 1, 503:512, :])

        def emit_mms(g, xlr, xpr, stat, pv, kp):
            # Stationary-major order: 4 weight loads per iteration.
            sv, svn, sdf, sdf2 = stat
            xs = ((xlr, 0), (xpr, 1024))
            for xx, c in xs:
                nc.tensor.matmul(g[0:pv, c:c + 512], sv[0:kp, 0:pv],
                                 xx[0:kp, 0:W], start=True, stop=False)
            for xx, c in xs:
                nc.tensor.matmul(g[0:pv, c:c + 512], svn[0:kp, 0:pv],
                                 xx[0:kp, 2:2 + W], start=False, stop=True)
            for xx, c in xs:
                nc.tensor.matmul(g[0:pv, c + 512:c + 1024], sdf[0:kp, 0:pv],
                                 xx[0:kp, 0:W], start=True, stop=False)
                nc.tensor.matmul(g[0:pv, c + 512:c + 1024], sdf[0:kp, 0:pv],
                                 xx[0:kp, 2:2 + W], start=False, stop=False)
            for xx, c in xs:
                nc.tensor.matmul(g[0:pv, c + 512:c + 1024], sdf2[0:kp, 0:pv],
                                 xx[0:kp, 1:1 + W], start=False, stop=True)

        loop_ctx = tc.For_i(0, loop, 1) if loop > 1 else None
        if loop_ctx is not None:
            loop_ctx.__enter__()

        emit_dmas()

        # Deferred per-iteration stages, emitted with a lag so engines
        # never wait on each other within an iteration.
        pending = []          # (sq, pv, it) waiting for madd/qmul
        qtiles = {}           # batch index -> q tile
        qfill = {}            # batch index -> number of filled slots

        def do_madd_qmul(sq, pv, it):
            m = m_pool.tile([128, 1024], F16)
            sqv = sq.rearrange("p (a b c) -> p a b c", a=2, b=2, c=512)
            mv = m.rearrange("p (a c) -> p a c", a=2, c=512)
            nc.vector.tensor_add(mv[0:pv, :, :], sqv[0:pv, :, 0, :],
                                 sqv[0:pv, :, 1, :])
            b, slot = divmod(it, QB)
            if slot == 0:
                qtiles[b] = q_pool.tile([128, QB * 512], F16, name=f"q{b}")
            q = qtiles[b]
            qfill[b] = slot + 1
            nc.vector.tensor_mul(q[0:pv, slot * 512:slot * 512 + 512],
                                 m[0:pv, 0:512], m[0:pv, 512:1024])
            if pv < 126:
                # zero unused partitions so the batched sqrt+accum over
                # [0:126] rows stays clean (packed-bottom iters, pv=32);
                # memset APs must start 32-aligned and span <= 32 partitions
                for p0, p1 in ((32, 64), (64, 96), (96, 126)):
                    nc.vector.memset(q[p0:p1, slot * 512:slot * 512 + 512], 0.0)

        def do_sqrt(b):
            q = qtiles.pop(b)
            w = qfill.pop(b) * 512
            nc.scalar.activation(q[0:126, 0:w], q[0:126, 0:w],
                                 mybir.ActivationFunctionType.Sqrt,
                                 accum_out=acc_b[0:126, b:b + 1])

        it = 0
        for phase in range(N_IMG + 2):
            if phase < N_IMG:
                img = phase
                bands = range(N_BANDS)
            else:
                bands = (-1,)
            for t in bands:
                if t >= 0:
                    xlr = blk(img, 0, t)
                    xpr = blk(img, 1, t)
                    stat, pv, kp = (BV, BVN, BDF, BDF2), BAND, 128
                else:
                    q2 = phase - N_IMG
                    xlr, xpr = xm[2 * q2], xm[2 * q2 + 1]
                    stat, pv, kp = (BVM, BVNM, BDFM, BDF2M), 32, 58

                # PSUM layout: [gx_l | gy_l | gx_p | gy_p], 512 f32 each.
                g = psum_pool.tile([128, 2048], F32)
                emit_mms(g, xlr, xpr, stat, pv, kp)

                # Eviction: ACT squares cols [0:ACT_SQ_COLS] (one batched
                # op), DVE copies + STT-squares the rest. Both accumulate
                # their share of sum(gx^2+gy^2) via accum_out.
                sq = sq_pool.tile([128, 2048], F16)
                nc.scalar.activation(sq[0:pv, 0:ACT_SQ_COLS],
                                     g[0:pv, 0:ACT_SQ_COLS],
                                     mybir.ActivationFunctionType.Square,
                                     accum_out=acc_a[0:pv, it:it + 1])
                dc = 2048 - ACT_SQ_COLS
                c16 = c16_pool.tile([128, dc], F16)
                nc.vector.tensor_copy(c16[0:pv, :], g[0:pv, ACT_SQ_COLS:2048])
                nc.vector.scalar_tensor_tensor(
                    out=sq[0:pv, ACT_SQ_COLS:2048], in0=c16[0:pv, :],
                    scalar=1.0, in1=c16[0:pv, :],
                    op0=mybir.AluOpType.mult, op1=mybir.AluOpType.mult,
                    accum_out=acc_c[0:pv, it:it + 1])

                # Lagged DVE madd/qmul (previous iteration's sq).
                pending.append((sq, pv, it))
                if len(pending) > 1:
                    do_madd_qmul(*pending.pop(0))
                # Lagged ACT sqrt over completed q batches.
                bq = (it - QB - SQRT_LAG) // QB
                if it % QB == SQRT_LAG and bq >= 0 and bq in qtiles:
                    do_sqrt(bq)
                it += 1

        while pending:
            do_madd_qmul(*pending.pop(0))
        for b in sorted(qtiles):
            do_sqrt(b)

        if loop_ctx is not None:
            loop_ctx.__exit__(None, None, None)
        nc.vector.tensor_reduce(out_s[:, 0:1], acc_a[:, :],
                                axis=mybir.AxisListType.X, op=mybir.AluOpType.add)
        nc.vector.tensor_reduce(out_s[:, 1:2], acc_b[:, :],
                                axis=mybir.AxisListType.X, op=mybir.AluOpType.add)
        nc.vector.tensor_reduce(out_s[:, 2:3], acc_c[:, :],
                                axis=mybir.AxisListType.X, op=mybir.AluOpType.add)
        nc.sync.dma_start(out=out[:, :], in_=out_s[:, :])
    return _patch_serialization(nc)


_NC = None


def kernel(probs, labels):
    global _NC
    from concourse.bass_utils import run_bass_kernel_spmd

    if _NC is None:
        _NC = build_kernel()

    p = np.asarray(probs)[:, 1:5].astype(np.float16)
    l = np.asarray(labels)[:, 1:5].astype(np.float16)
    wmat = _stationaries()

    in_maps = []
    for k in range(8):
        # x[i, 0] = labels image i, x[i, 1] = probs image i
        xi = np.stack([l[2 * k:2 * k + 2].reshape(N_IMG, H, W),
                       p[2 * k:2 * k + 2].reshape(N_IMG, H, W)], axis=1)
        in_maps.append({
            "x": np.ascontiguousarray(xi),
            "consts": wmat,
        })
    res = run_bass_kernel_spmd(_NC, in_maps, list(range(8)))
    total = 0.0
    for r in res.results:
        o = r["out"].astype(np.float64)
        total += o[:, 0].sum() + o[:, 2].sum() - 2.0 * o[:, 1].sum()
    return np.float32(total / (16 * H * W))


# revision 6
# speedup vs baseline: 1.2460x; 1.1526x over previous
"""BoundaryLoss kernel for 8 Trainium2 NeuronCores (v3).

loss = sum_c mean_{b,h,w}((|sobel(labels_c)| - |sobel(probs_c)|)^2)

Data-parallel: core k processes batches [2k, 2k+1] x classes 1..4
(8 image pairs of 512x512). Per-core partial sums are combined on host.

v3: the per-dma_start fixed cost (~2-3us, serialized per HWDGE ring)
dominated earlier versions (baseline: 84 DMAs ~= 170us). The host now
pre-packs SBUF-layout pair tiles (row-band blocks with halo rows and
zero pads), so the device issues only 9 big contiguous DMAs per loop
iteration, split across the two HWDGE rings (nc.sync / nc.scalar).

Per-iteration compute (measured-cost-balanced):
  - TensorE: 10 fp16 band-matrix matmuls -> PSUM [gx_l|gy_l|gx_p|gy_p].
  - ACT: one batched Square over PSUM cols [0:ACT_SQ_COLS] (+accum).
  - DVE: copy PSUM cols [ACT_SQ_COLS:2048] -> fp16, STT square (+accum).
  - DVE (lagged 1 iter): m = gx^2+gy^2 (one TT), q = m_l*m_p into a
    4-iter q batch.
  - ACT (lagged, per 4 iters): Sqrt over the q batch (+accum).
  loss*B*H*W = sum(acc_a) + sum(acc_c) - 2*sum(acc_b), combined on host.
"""

import sys

import numpy as np

if "/opt/trn_rl_repo" not in sys.path:
    sys.path.insert(0, "/opt/trn_rl_repo")

from contextlib import ExitStack

import concourse.bass as bass
import concourse.mybir as mybir
import concourse.tile as tile

H = W = 512
N_IMG = 8          # image pairs per core
BAND = 126         # output rows per full band
N_BANDS = 4        # full 126-row bands; bottom 8 rows via 2 packed iters
N_ITERS = N_IMG * N_BANDS + 2
PADW = W + 2       # padded columns per block
BLK = PADW         # block stride inside a pair tile
PAIRW = 2 * 4 * BLK  # columns per pair tile (2 sides x 4 blocks)
SMOOTH = 1e-6
# columns of the 2048-wide PSUM handled by ACT Square (rest via DVE)
ACT_SQ_COLS = 1472
QB = 4             # iters per sqrt batch
SQRT_LAG = 2       # sqrt of batch b is emitted during iter 4*b+4+SQRT_LAG

F32 = mybir.dt.float32
F16 = mybir.dt.float16


def _stationaries():
    """lhsT weight matrices [p, c]: moving partition p -> out partition c."""
    bv = np.zeros((128, 128), np.float32)   # vertical smooth [1,2,1]
    bdf = np.zeros((128, 128), np.float32)  # vertical diff [1,0,-1]
    for c in range(126):
        bv[c, c] = 1.0
        bv[c + 1, c] = 2.0
        bv[c + 2, c] = 1.0
        bdf[c, c] = 1.0
        bdf[c + 2, c] = -1.0
    # Packed bottom-band versions: 4 images per iteration; image k's rows
    # 503..511 live at input partitions 16k..16k+8 (16k+9 is the zeroed
    # row-512 halo), outputs 504..511 at partitions 8k..8k+7.
    bvm = np.zeros((128, 128), np.float32)
    bdfm = np.zeros((128, 128), np.float32)
    for k in range(4):
        for i in range(8):
            bvm[16 * k + i, 8 * k + i] = 1.0
            bvm[16 * k + i + 1, 8 * k + i] = 2.0
            bvm[16 * k + i + 2, 8 * k + i] = 1.0
            bdfm[16 * k + i, 8 * k + i] = 1.0
            bdfm[16 * k + i + 2, 8 * k + i] = -1.0
    return np.concatenate(
        [bv, -bv, bdf, 2.0 * bdf, bvm, -bvm, bdfm, 2.0 * bdfm],
        axis=1).astype(np.float16)


def pack_host_inputs(l4, p4):
    """Build the SBUF-layout host tensors.

    l4, p4: float16 [8, 512, 512] (labels / probs images for this core).
    Returns xs [8, 128, PAIRW] and xmh [128, 4 * BLK].
    """
    xs = np.zeros((N_IMG, 128, PAIRW), np.float16)
    for i in range(N_IMG):
        for s, img in ((0, l4[i]), (1, p4[i])):
            c0 = (4 * s) * BLK
            xs[i, 1:128, c0 + 1:c0 + 1 + W] = img[0:127]
            for b in range(1, 4):
                cb = (4 * s + b) * BLK
                r0 = 126 * b - 1
                xs[i, :, cb + 1:cb + 1 + W] = img[r0:r0 + 128]
    xmh = np.zeros((128, 4 * BLK), np.float16)
    for q in range(2):
        for s, arr in ((0, l4), (1, p4)):
            j = 2 * q + s
            for k in range(4):
                xmh[16 * k:16 * k + 9, j * BLK + 1:j * BLK + 1 + W] = \
                    arr[4 * q + k, 503:512]
    return xs, xmh


def _split_waits_json(bir: bytes, maxw: int = 1) -> bytes:
    """Walrus in this container rejects instructions with >1 semaphore wait
    ("Too many sync wait commands"). Split extra waits onto NoOp carriers
    inserted just before the instruction on the same engine — semantics are
    identical (same waits, same order, before the instruction executes)."""
    import orjson

    d = orjson.loads(bir)
    ctr = 0
    for fn in d["functions"]:
        for b in fn["blocks"]:
            new = []
            for ins in b["instructions"]:
                si = ins.get("sync_info")
                if si:
                    waits = si.get("on_wait") or []
                    if len(waits) > maxw:
                        keep = waits[-maxw:] if maxw else []
                        for w in waits[: len(waits) - maxw]:
                            ctr += 1
                            new.append({
                                "debug": ins.get("debug", 0),
                                "engine": ins["engine"],
                                "ins": [],
                                "outs": [],
                                "name": f"{ins['name']}-wsplit{ctr}",
                                "opcode": "NoOp",
                                "sync_info": {"on_wait": [w], "on_update": []},
                            })
                        si["on_wait"] = keep
                new.append(ins)
            b["instructions"] = new
    return orjson.dumps(d)


def _patch_serialization(nc):
    fixed = _split_waits_json(nc.to_json_bytes())
    nc.to_json_bytes = lambda: fixed
    return nc


def build_kernel(loop: int = 1, variant: str = "full"):
    nc = bass.Bass()
    xs = nc.dram_tensor("xs", [N_IMG, 128, PAIRW], F16, kind="ExternalInput")
    xmh = nc.dram_tensor("xmh", [128, 4 * BLK], F16, kind="ExternalInput")
    consts = nc.dram_tensor("consts", [128, 1024], F16, kind="ExternalInput")
    out = nc.dram_tensor("out", [128, 3], F32, kind="ExternalOutput")

    with ExitStack() as ctx:
        tc = ctx.enter_context(tile.TileContext(nc))
        cpool = ctx.enter_context(tc.tile_pool(name="consts", bufs=1))
        xpool = ctx.enter_context(tc.tile_pool(name="x", bufs=1))
        psum_pool = ctx.enter_context(tc.tile_pool(name="g", bufs=2, space="PSUM"))
        sq_pool = ctx.enter_context(tc.tile_pool(name="sq", bufs=4))
        c16_pool = ctx.enter_context(tc.tile_pool(name="c16", bufs=4))
        m_pool = ctx.enter_context(tc.tile_pool(name="m", bufs=4))
        q_pool = ctx.enter_context(tc.tile_pool(name="q", bufs=3))
        acc_pool = ctx.enter_context(tc.tile_pool(name="acc", bufs=1))

        wmat = cpool.tile([128, 1024], F16, tag="wmat")
        nc.sync.dma_start(out=wmat[:, :], in_=consts[:, :])
        (BV, BVN, BDF, BDF2, BVM, BVNM, BDFM, BDF2M) = (
            wmat[:, 128 * i:128 * i + 128] for i in range(8))

        acc_a = acc_pool.tile([128, N_ITERS], F32, tag="acc_a")
        acc_c = acc_pool.tile([128, N_ITERS], F32, tag="acc_c")
        acc_b = acc_pool.tile([128, N_ITERS // QB + 1], F32, tag="acc_b")
        nc.vector.memset(acc_a[:, :], 0.0)
        nc.vector.memset(acc_b[:, :], 0.0)
        nc.vector.memset(acc_c[:, :], 0.0)
        out_s = acc_pool.tile([128, 3], F32, tag="out_s")

        # One mega-tile holding all 8 pair tiles + one packed-bottom tile.
        # Layout comes pre-built from the host (halos, zero pads included).
        xall = xpool.tile([128, N_IMG * PAIRW], F16, tag="xall")
        xmt = xpool.tile([128, 4 * BLK], F16, tag="xmt")

        def blk(i, s, b):
            """Block AP [128, BLK] of pair i, side s, block b."""
            c = i * PAIRW + (4 * s + b) * BLK
            return xall[:, c:c + BLK]

        def emit_dmas():
            # pair i on ring (sync / scalar) alternating; xm last on scalar
            for i in range(N_IMG):
                eng = nc.sync if i % 2 == 0 else nc.scalar
                eng.dma_start(
                    out=xall[:, i * PAIRW:(i + 1) * PAIRW],
                    in_=xs[i, :, :])
            nc.scalar.dma_start(out=xmt[:, :], in_=xmh[:, :])

        def emit_mms(g, xlr, xpr, stat, pv, kp):
            # Stationary-major order: 4 weight loads per iteration.
            sv, svn, sdf, sdf2 = stat
            xx = ((xlr, 0), (xpr, 1024))
            for x_, c in xx:
                nc.tensor.matmul(g[0:pv, c:c + 512], sv[0:kp, 0:pv],
                                 x_[0:kp, 0:W], start=True, stop=False)
            for x_, c in xx:
                nc.tensor.matmul(g[0:pv, c:c + 512], svn[0:kp, 0:pv],
                                 x_[0:kp, 2:2 + W], start=False, stop=True)
            for x_, c in xx:
                nc.tensor.matmul(g[0:pv, c + 512:c + 1024], sdf[0:kp, 0:pv],
                                 x_[0:kp, 0:W], start=True, stop=False)
                nc.tensor.matmul(g[0:pv, c + 512:c + 1024], sdf[0:kp, 0:pv],
                                 x_[0:kp, 2:2 + W], start=False, stop=False)
            for x_, c in xx:
                nc.tensor.matmul(g[0:pv, c + 512:c + 1024], sdf2[0:kp, 0:pv],
                                 x_[0:kp, 1:1 + W], start=False, stop=True)

        loop_ctx = tc.For_i(0, loop, 1) if loop > 1 else None
        if loop_ctx is not None:
            loop_ctx.__enter__()

        emit_dmas()

        # Deferred per-iteration stages, emitted with a lag so engines
        # never wait on each other within an iteration.
        pending = []          # (sq, pv, it) waiting for madd/qmul
        qtiles = {}           # batch index -> q tile
        qfill = {}            # batch index -> number of filled slots

        def do_madd_qmul(sq, pv, it):
            m = m_pool.tile([128, 1024], F16)
            sqv = sq.rearrange("p (a b c) -> p a b c", a=2, b=2, c=512)
            mv = m.rearrange("p (a c) -> p a c", a=2, c=512)
            nc.vector.tensor_add(mv[0:pv, :, :], sqv[0:pv, :, 0, :],
                                 sqv[0:pv, :, 1, :])
            b, slot = divmod(it, QB)
            if slot == 0:
                qtiles[b] = q_pool.tile([128, QB * 512], F16, name=f"q{b}")
            q = qtiles[b]
            qfill[b] = slot + 1
            nc.vector.tensor_mul(q[0:pv, slot * 512:slot * 512 + 512],
                                 m[0:pv, 0:512], m[0:pv, 512:1024])
            if pv < 126:
                # zero unused partitions so the batched sqrt+accum over
                # [0:126] rows stays clean (packed-bottom iters, pv=32);
                # memset APs must start 32-aligned and span <= 32 partitions
                for p0, p1 in ((32, 64), (64, 96), (96, 126)):
                    nc.vector.memset(q[p0:p1, slot * 512:slot * 512 + 512], 0.0)

        def do_sqrt(b):
            q = qtiles.pop(b)
            w = qfill.pop(b) * 512
            nc.scalar.activation(q[0:126, 0:w], q[0:126, 0:w],
                                 mybir.ActivationFunctionType.Sqrt,
                                 accum_out=acc_b[0:126, b:b + 1])

        it = 0
        for phase in range(N_IMG + 2):
            if phase < N_IMG:
                img = phase
                bands = range(N_BANDS)
            else:
                bands = (-1,)
            for t in bands:
                if t >= 0:
                    xlr = blk(img, 0, t)
                    xpr = blk(img, 1, t)
                    stat, pv, kp = (BV, BVN, BDF, BDF2), BAND, 128
                else:
                    q2 = phase - N_IMG
                    xlr = xmt[:, (2 * q2) * BLK:(2 * q2) * BLK + BLK]
                    xpr = xmt[:, (2 * q2 + 1) * BLK:(2 * q2 + 1) * BLK + BLK]
                    stat, pv, kp = (BVM, BVNM, BDFM, BDF2M), 32, 58

                if variant == "dma":
                    it += 1
                    continue
                # PSUM layout: [gx_l | gy_l | gx_p | gy_p], 512 f32 each.
                g = psum_pool.tile([128, 2048], F32)
                emit_mms(g, xlr, xpr, stat, pv, kp)

                if variant == "dma_mm":
                    sqm = sq_pool.tile([128, 2048], F16)
                    nc.vector.tensor_copy(sqm[0:4, :], g[0:4, :])
                    nc.vector.tensor_reduce(
                        acc_a[0:4, it:it + 1], sqm[0:4, :],
                        axis=mybir.AxisListType.X, op=mybir.AluOpType.add)
                    it += 1
                    continue
                # Eviction: ACT squares cols [0:ACT_SQ_COLS] (one batched
                # op), DVE copies + STT-squares the rest. Both accumulate
                # their share of sum(gx^2+gy^2) via accum_out.
                sq = sq_pool.tile([128, 2048], F16)
                nc.scalar.activation(sq[0:pv, 0:ACT_SQ_COLS],
                                     g[0:pv, 0:ACT_SQ_COLS],
                                     mybir.ActivationFunctionType.Square,
                                     accum_out=acc_a[0:pv, it:it + 1])
                dc = 2048 - ACT_SQ_COLS
                c16 = c16_pool.tile([128, dc], F16)
                nc.vector.tensor_copy(c16[0:pv, :], g[0:pv, ACT_SQ_COLS:2048])
                nc.vector.scalar_tensor_tensor(
                    out=sq[0:pv, ACT_SQ_COLS:2048], in0=c16[0:pv, :],
                    scalar=1.0, in1=c16[0:pv, :],
                    op0=mybir.AluOpType.mult, op1=mybir.AluOpType.mult,
                    accum_out=acc_c[0:pv, it:it + 1])

                # Lagged DVE madd/qmul (previous iteration's sq).
                pending.append((sq, pv, it))
                if len(pending) > 1:
                    do_madd_qmul(*pending.pop(0))
                # Lagged ACT sqrt over completed q batches.
                bq = (it - QB - SQRT_LAG) // QB
                if it % QB == SQRT_LAG and bq >= 0 and bq in qtiles:
                    do_sqrt(bq)
                it += 1

        while pending:
            do_madd_qmul(*pending.pop(0))
        for b in sorted(qtiles):
            do_sqrt(b)
        if variant == "dma":
            # consume the tiles so the DMAs stay live
            for i in range(N_IMG):
                nc.vector.tensor_reduce(
                    acc_a[:, i:i + 1], xall[:, i * PAIRW:i * PAIRW + 512],
                    axis=mybir.AxisListType.X, op=mybir.AluOpType.add)
            nc.vector.tensor_reduce(acc_a[:, 8:9], xmt[:, :],
                                    axis=mybir.AxisListType.X,
                                    op=mybir.AluOpType.add)

        if loop_ctx is not None:
            loop_ctx.__exit__(None, None, None)
        nc.vector.tensor_reduce(out_s[:, 0:1], acc_a[:, :],
                                axis=mybir.AxisListType.X, op=mybir.AluOpType.add)
        nc.vector.tensor_reduce(out_s[:, 1:2], acc_b[:, :],
                                axis=mybir.AxisListType.X, op=mybir.AluOpType.add)
        nc.vector.tensor_reduce(out_s[:, 2:3], acc_c[:, :],
                                axis=mybir.AxisListType.X, op=mybir.AluOpType.add)
        nc.sync.dma_start(out=out[:, :], in_=out_s[:, :])
    return _patch_serialization(nc)


_NC = None


def kernel(probs, labels):
    global _NC
    from concourse.bass_utils import run_bass_kernel_spmd

    if _NC is None:
        _NC = build_kernel()

    p = np.asarray(probs)[:, 1:5].astype(np.float16)
    l = np.asarray(labels)[:, 1:5].astype(np.float16)
    wmat = _stationaries()

    in_maps = []
    for k in range(8):
        l4 = l[2 * k:2 * k + 2].reshape(N_IMG, H, W)
        p4 = p[2 * k:2 * k + 2].reshape(N_IMG, H, W)
        xs_h, xm_h = pack_host_inputs(l4, p4)
        in_maps.append({"xs": xs_h, "xmh": xm_h, "consts": wmat})
    res = run_bass_kernel_spmd(_NC, in_maps, list(range(8)))
    total = 0.0
    for r in res.results:
        o = r["out"].astype(np.float64)
        total += o[:, 0].sum() + o[:, 2].sum() - 2.0 * o[:, 1].sum()
    return np.float32(total / (16 * H * W))
